# revision 26
# baseline (speedup 1.0000x reference)
# Trainium2 Bass kernel for CenterLoss (nn_CenterLoss_83090437308894).
#
# Strategy (per sharding hint): shard the 100000x512 centers table row-wise
# across 8 NeuronCores (12500 rows each). Each core bulk-copies its shard to
# the output (the memory-bound part: 25.6MB read + 25.6MB write of HBM per
# core) while redundantly computing the small (B,B) pairwise-center math:
#   dist^2 = rn_i + rn_j - 2*cg@cg^T        (PE matmuls)
#   E = exp(-dist) * mask                    (symmetric => reusable as lhsT)
#   delta2 = cS_i*cg_i - c_i*(E@cg)_i        (PE matmul + row scaling)
#   new_vals = 0.5*cg + 0.5*feat - 0.1*delta2, loss = mean(clip((cg-f)^2))
# Host does the cheap index work: gather cg = centers[labels] (1MB), the
# label inequality mask, and the final scatter of the <=512 updated rows
# (last-write-wins dedup) into the gathered output.
import os
import sys
import numpy as np

B, D, C = 512, 512, 100000
NCORES = 8
SHARD = C // NCORES          # 12500 rows per core
MARGIN = 50.0
SCALE = 1.0
NCHUNK = 10                  # bulk-copy chunks per core
P = 128
NB = B // P                  # 4 row blocks of 128

_PROGRAM = None
LAST_RESULT = None

for _p in ("/opt/trn_rl_repo", "/root/.axon_site/_ro/trn_rl_repo"):
    if os.path.isdir(_p) and _p not in sys.path:
        sys.path.append(_p)


def _build_program():
    from contextlib import ExitStack

    import concourse.bacc as bacc
    import concourse.tile as tile
    from concourse import mybir

    f32 = mybir.dt.float32
    AF = mybir.ActivationFunctionType
    ALU = mybir.AluOpType
    AX = mybir.AxisListType

    bf16 = mybir.dt.bfloat16
    from concourse.masks import make_identity

    nc = bacc.Bacc(None, target_bir_lowering=False)
    shard = nc.dram_tensor("shard", [SHARD, D], f32, kind="ExternalInput")
    cg_d = nc.dram_tensor("cg", [B, D], f32, kind="ExternalInput")
    f_d = nc.dram_tensor("feat", [B, D], f32, kind="ExternalInput")
    labr_d = nc.dram_tensor("labr", [1, B], f32, kind="ExternalInput")
    labc_d = nc.dram_tensor("labc", [P, NB], f32, kind="ExternalInput")
    out_shard = nc.dram_tensor("out_shard", [SHARD, D], f32, kind="ExternalOutput")
    nv_d = nc.dram_tensor("new_vals", [B, D], f32, kind="ExternalOutput")
    loss_d = nc.dram_tensor("loss", [1, 1], f32, kind="ExternalOutput")

    with ExitStack() as ctx:
        tc = ctx.enter_context(tile.TileContext(nc))
        big = ctx.enter_context(tc.tile_pool(name="big", bufs=1))
        scr = ctx.enter_context(tc.tile_pool(name="scr", bufs=4))
        st = ctx.enter_context(tc.tile_pool(name="st", bufs=1))
        pbig = ctx.enter_context(tc.tile_pool(name="pbig", bufs=4, space="PSUM"))
        psml = ctx.enter_context(tc.tile_pool(name="psml", bufs=2, space="PSUM"))

        # ---- input loads FIRST on the sync HWDGE ring (FIFO per ring), so
        # the compute chain isn't starved behind the bulk copy's packets ----
        CG = big.tile([P, NB, D], f32)      # cg     (i on partitions, d free)
        F = big.tile([P, NB, D], f32)
        labr = st.tile([1, B], f32)         # labels as f32, row layout
        labc = st.tile([P, NB], f32)        # labels as f32, (c p) layout
        nc.sync.dma_start(out=CG[:, :, :], in_=cg_d[:, :].rearrange("(c p) d -> p c d", p=P))
        nc.sync.dma_start(out=F[:, :, :], in_=f_d[:, :].rearrange("(c p) d -> p c d", p=P))
        nc.sync.dma_start(out=labr, in_=labr_d[:, :])
        nc.sync.dma_start(out=labc, in_=labc_d[:, :])

        # ---- bulk shard copy (DRAM->DRAM) behind the loads on the same
        # HWDGE ring (FIFO): loads finish first, copy then streams at the
        # ring's full rate. Splitting across rings/SWDGE was measured
        # slower (no aggregate gain + it starves the small load packets). ----
        rows = SHARD // NCHUNK
        assert rows * NCHUNK == SHARD
        shard_f = shard[:, :].rearrange("r d -> (r d)")
        out_f = out_shard[:, :].rearrange("r d -> (r d)")
        celts = rows * D
        for k in range(NCHUNK):
            nc.sync.dma_start(
                out=out_f[k * celts:(k + 1) * celts],
                in_=shard_f[k * celts:(k + 1) * celts],
            )

        ones_p1 = nc.const_aps.tensor(1.0, (P, 1), f32)  # preamble const: dep-free
        ones_1p = st.tile([1, P], f32)
        nc.vector.memset(ones_1p, 1.0)

        # ---- cg^T on device via PE transpose (saves a 1MB HBM load) ----
        ident = st.tile([P, P], f32)
        make_identity(nc, ident)
        CT = big.tile([P, NB, D], f32)      # cg^T   (d on partitions, i free)
        CTn2 = big.tile([P, NB, D], f32)    # -2 * cg^T
        CG2 = big.tile([P, NB, D], bf16)    # bf16 cg for the E@cg matmul
        SQA = big.tile([P, NB, D], f32)     # (cg^T)^2
        CGSQ = big.tile([P, NB, D], f32)    # cg^2 (row-norm source)
        E = big.tile([P, NB, B], bf16)      # exp(-dist)*mask (symmetric)
        for c2 in range(NB):
            ptr = pbig.tile([P, B], f32, tag="mm")
            for im in range(NB):
                nc.tensor.transpose(ptr[:, im * P:(im + 1) * P],
                                    CG[:, im, c2 * P:(c2 + 1) * P], ident)
            nc.vector.tensor_copy(CT[:, c2, :], ptr)
            nc.vector.tensor_scalar(CTn2[:, c2, :], ptr, -2.0, None, op0=ALU.mult)
            nc.vector.tensor_mul(SQA[:, c2, :], CT[:, c2, :], CT[:, c2, :])
        nc.vector.tensor_copy(CG2[:, :, :], CG[:, :, :])
        nc.vector.tensor_mul(CGSQ[:, :, :], CG[:, :, :], CG[:, :, :])

        # ---- loss partials early: they only need CG/F, so DVE fills its
        # idle windows with them instead of tailing past the bulk copy ----
        ls4 = st.tile([P, NB], f32)     # per-row sum(clip((cg-f)^2))
        red2 = st.tile([P, 2], f32)
        for im in range(NB):
            df = scr.tile([P, D], f32, tag="df")
            nc.vector.tensor_sub(df, CG[:, im, :], F[:, im, :])
            sq2 = scr.tile([P, D], f32, tag="sq2")
            nc.vector.tensor_mul(sq2, df, df)
            cl = scr.tile([P, D], f32, tag="cl")
            nc.vector.tensor_scalar(cl, sq2, 1e-12, 1e12, op0=ALU.max, op1=ALU.min)
            nc.vector.tensor_reduce(ls4[:, im:im + 1], cl, AX.X, ALU.add)
        nc.vector.tensor_reduce(red2[:, 1:2], ls4, AX.X, ALU.add)
        pls = psml.tile([1, 1], f32, tag="tiny")
        nc.tensor.matmul(pls, lhsT=red2[:, 1:2], rhs=ones_p1, start=True, stop=True)
        loss_sb = st.tile([1, 1], f32)
        nc.vector.tensor_scalar(loss_sb, pls, float(SCALE / (B * D)), None, op0=ALU.mult)
        nc.gpsimd.dma_start(out=loss_d[:, :], in_=loss_sb)

        rn_i4 = st.tile([P, NB], f32)       # per-row norms, i on partitions
        for c in range(NB):
            nc.vector.tensor_reduce(rn_i4[:, c:c + 1], CGSQ[:, c, :], AX.X, ALU.add)

        # ---- rnT[1, j] = sum_d cg[j, d]^2 via ones^T @ (cg^T)^2 ----
        prnT = psml.tile([1, B], f32, tag="seq")
        for c in range(NB):
            nc.tensor.matmul(prnT, lhsT=ones_p1, rhs=SQA[:, c, :], start=(c == 0), stop=(c == NB - 1))
        rnT = st.tile([1, B], f32)
        nc.vector.tensor_copy(rnT, prnT)
        # broadcast rn_j across partitions once (K=1 matmul), keep in SBUF
        prnb = psml.tile([P, B], f32, tag="seq")
        nc.tensor.matmul(prnb, lhsT=ones_1p, rhs=rnT, start=True, stop=True)
        rnj = big.tile([P, B], f32)
        nc.vector.tensor_copy(rnj, prnb)
        # broadcast labels row across partitions (for the on-device neq mask)
        plab = psml.tile([P, B], f32, tag="seq")
        nc.tensor.matmul(plab, lhsT=ones_1p, rhs=labr, start=True, stop=True)
        labj = big.tile([P, B], f32)
        nc.vector.tensor_copy(labj, plab)

        mx4 = st.tile([P, NB], f32)     # per-row max(dist*m)  (= -min_v)
        S4 = st.tile([P, NB], f32)      # per-row sum(E)
        msr4 = st.tile([P, NB], f32)    # per-row sum(m)

        # ---- phase 1, staged across all 4 row blocks so ACT runs
        # same-function batches (2 table loads: Sqrt then Exp) ----
        pgs = []
        for im in range(NB):
            isl = slice(im * P, (im + 1) * P)
            pg = pbig.tile([P, B], f32, tag="mm")
            for dk in range(NB):
                nc.tensor.matmul(pg, lhsT=CTn2[:, dk, isl], rhs=CT[:, dk, :],
                                 start=(dk == 0), stop=(dk == NB - 1))
            pgs.append(pg)
        d2s, dists = [], []
        for im in range(NB):
            raw = scr.tile([P, B], f32, tag="raw")
            nc.vector.scalar_tensor_tensor(raw, pgs[im], rn_i4[:, im:im + 1], rnj,
                                           op0=ALU.add, op1=ALU.add)
            d2 = scr.tile([P, B], f32, tag="d2")
            nc.vector.tensor_scalar(d2, raw, 0.0, None, op0=ALU.max)
            d2s.append(d2)
        for im in range(NB):
            dist = scr.tile([P, B], f32, tag="dist")
            nc.scalar.activation(out=dist, in_=d2s[im], func=AF.Sqrt)
            dists.append(dist)
        ets = []
        for im in range(NB):
            e_t = scr.tile([P, B], f32, tag="et")
            nc.scalar.activation(out=e_t, in_=dists[im], func=AF.Exp, scale=-1.0)
            ets.append(e_t)
        for im in range(NB):
            dist = dists[im]
            le_t = scr.tile([P, B], f32, tag="le")
            nc.vector.tensor_scalar(le_t, dist, float(MARGIN), None, op0=ALU.is_le)
            m_t = scr.tile([P, B], f32, tag="mt")
            # m = (lab_j != lab_i) * (dist <= margin)
            nc.vector.scalar_tensor_tensor(m_t, labj, labc[:, im:im + 1], le_t,
                                           op0=ALU.not_equal, op1=ALU.mult)
            nc.vector.tensor_mul(E[:, im, :], ets[im], m_t)
            dm = scr.tile([P, B], f32, tag="dm")
            nc.vector.tensor_mul(dm, dist, m_t)
            nc.vector.tensor_reduce(mx4[:, im:im + 1], dm, AX.X, ALU.max)
            nc.vector.tensor_reduce(S4[:, im:im + 1], E[:, im, :], AX.X, ALU.add)
            nc.vector.tensor_reduce(msr4[:, im:im + 1], m_t, AX.X, ALU.add)

        # ---- per-row softmax factors ----
        em4 = st.tile([P, NB], f32)
        nc.scalar.activation(out=em4, in_=mx4, func=AF.Exp)
        z4 = st.tile([P, NB], f32)
        nc.vector.tensor_mul(z4, em4, S4)
        nc.vector.tensor_scalar(z4, z4, 1e-6, None, op0=ALU.add)
        rz4 = st.tile([P, NB], f32)
        nc.vector.reciprocal(rz4, z4)
        c4 = st.tile([P, NB], f32)
        nc.vector.tensor_mul(c4, em4, rz4)
        cS4 = st.tile([P, NB], f32)
        nc.vector.tensor_mul(cS4, c4, S4)

        # ---- msum gate (partition reduce via matmul) ----
        nc.vector.tensor_reduce(red2[:, 0:1], msr4, AX.X, ALU.add)
        pms = psml.tile([1, 1], f32, tag="tiny")
        nc.tensor.matmul(pms, lhsT=red2[:, 0:1], rhs=ones_p1, start=True, stop=True)
        gate = st.tile([1, 1], f32)
        nc.vector.tensor_scalar(gate, pms, 1.0, None, op0=ALU.is_ge)
        pgb = psml.tile([P, 1], f32, tag="tiny")
        nc.tensor.matmul(pgb, lhsT=ones_1p, rhs=gate, start=True, stop=True)
        gb = st.tile([P, 1], f32)
        nc.vector.tensor_copy(gb, pgb)

        # p1 = 0.5 - 0.1*g*cS ; p2 = 0.1*g*c  (per row)
        p14 = st.tile([P, NB], f32)
        nc.vector.tensor_scalar(p14, cS4, gb, -0.1, op0=ALU.mult, op1=ALU.mult)
        nc.vector.tensor_scalar(p14, p14, 0.5, None, op0=ALU.add)
        p24 = st.tile([P, NB], f32)
        nc.vector.tensor_scalar(p24, c4, gb, 0.1, op0=ALU.mult, op1=ALU.mult)

        # ---- phase 2: R = E @ cg (bf16) ; new_vals = p1*cg + 0.5*feat + p2*R ----
        for im in range(NB):
            isl = slice(im * P, (im + 1) * P)
            pr = pbig.tile([P, D], f32, tag="mm")
            for jk in range(NB):
                nc.tensor.matmul(pr, lhsT=E[:, jk, isl], rhs=CG2[:, jk, :],
                                 start=(jk == 0), stop=(jk == NB - 1))
            t1 = scr.tile([P, D], f32, tag="t1")
            nc.vector.tensor_scalar(t1, CG[:, im, :], p14[:, im:im + 1], None, op0=ALU.mult)
            t2 = scr.tile([P, D], f32, tag="t2")
            nc.vector.tensor_scalar(t2, F[:, im, :], 0.5, None, op0=ALU.mult)
            nc.vector.tensor_add(t1, t1, t2)
            t4 = scr.tile([P, D], f32, tag="t4")
            nc.vector.tensor_scalar(t4, pr, p24[:, im:im + 1], None, op0=ALU.mult)
            nc.vector.tensor_add(t1, t1, t4)
            nc.gpsimd.dma_start(out=nv_d[isl, :], in_=t1)

    nc.compile()
    return nc


def _ensure_ntff_hook():
    """Register antenv.axon_hooks (missing in this image) so
    run_bass_kernel_spmd(trace=True) can NTFF-profile via libaxon_pjrt."""
    import types

    if "antenv.axon_hooks" in sys.modules:
        return
    try:
        import antenv
        from trn_agent_boot.trn_boot import _ntff_profile_via_ctypes

        hook = _ntff_profile_via_ctypes("/opt/axon/libaxon_pjrt.so")
        mod = types.ModuleType("antenv.axon_hooks")
        mod._hook = hook
        mod.get_axon_ntff_profile_hook = lambda: mod._hook

        def _set(h):
            mod._hook = h

        mod.set_axon_ntff_profile_hook = _set
        sys.modules["antenv.axon_hooks"] = mod
        antenv.axon_hooks = mod
    except Exception:
        pass


def kernel(features, labels, centers):
    global _PROGRAM, LAST_RESULT
    from concourse.bass_utils import run_bass_kernel_spmd

    features = np.ascontiguousarray(np.asarray(features, dtype=np.float32))
    labels_np = np.asarray(labels).astype(np.int32)
    centers = np.ascontiguousarray(np.asarray(centers, dtype=np.float32))

    cg = np.ascontiguousarray(centers[labels_np])
    labr = labels_np.astype(np.float32).reshape(1, B)
    labc = np.ascontiguousarray(labels_np.astype(np.float32).reshape(NB, P).T)

    if _PROGRAM is None:
        _PROGRAM = _build_program()
    nc = _PROGRAM

    in_maps = [
        {
            "shard": centers[c * SHARD:(c + 1) * SHARD],
            "cg": cg,
            "feat": features,
            "labr": labr,
            "labc": labc,
        }
        for c in range(NCORES)
    ]
    trace = bool(int(os.environ.get("KERNEL_TRACE", "0") or "0"))
    if trace:
        _ensure_ntff_hook()
    res = run_bass_kernel_spmd(nc, in_maps, core_ids=list(range(NCORES)), trace=trace)
    LAST_RESULT = res

    out = np.empty_like(centers)
    for c in range(NCORES):
        out[c * SHARD:(c + 1) * SHARD] = res.results[c]["out_shard"]
    nv = res.results[0]["new_vals"]
    # scatter updated rows; duplicates resolve to the last batch occurrence
    rev = labels_np[::-1]
    u, idx = np.unique(rev, return_index=True)
    out[u] = nv[(B - 1) - idx]
    loss = np.float32(res.results[0]["loss"][0, 0])
    return loss, out


# revision 27
# speedup vs baseline: 1.1344x; 1.1344x over previous
# Trainium2 Bass kernel for CenterLoss (nn_CenterLoss_83090437308894).
#
# Strategy (per sharding hint): shard the 100000x512 centers table row-wise
# across 8 NeuronCores (12500 rows each). Each core bulk-copies its shard to
# the output (the memory-bound part: 25.6MB read + 25.6MB write of HBM per
# core) while redundantly computing the small (B,B) pairwise-center math:
#   dist^2 = rn_i + rn_j - 2*cg@cg^T        (PE matmuls)
#   E = exp(-dist) * mask                    (symmetric => reusable as lhsT)
#   delta2 = cS_i*cg_i - c_i*(E@cg)_i        (PE matmul + row scaling)
#   new_vals = 0.5*cg + 0.5*feat - 0.1*delta2, loss = mean(clip((cg-f)^2))
# Host does the cheap index work: gather cg = centers[labels] (1MB), the
# label inequality mask, and the final scatter of the <=512 updated rows
# (last-write-wins dedup) into the gathered output.
import os
import sys
import numpy as np

B, D, C = 512, 512, 100000
NCORES = 8
SHARD = C // NCORES          # 12500 rows per core
MARGIN = 50.0
SCALE = 1.0
NCHUNK = 20                  # bulk-copy chunks per core
P = 128
NB = B // P                  # 4 row blocks of 128

_PROGRAM = None
LAST_RESULT = None

for _p in ("/opt/trn_rl_repo", "/root/.axon_site/_ro/trn_rl_repo"):
    if os.path.isdir(_p) and _p not in sys.path:
        sys.path.append(_p)


def _build_program():
    from contextlib import ExitStack

    import concourse.bacc as bacc
    import concourse.tile as tile
    from concourse import mybir

    f32 = mybir.dt.float32
    AF = mybir.ActivationFunctionType
    ALU = mybir.AluOpType
    AX = mybir.AxisListType

    bf16 = mybir.dt.bfloat16
    from concourse.masks import make_identity

    nc = bacc.Bacc(None, target_bir_lowering=False)
    shard = nc.dram_tensor("shard", [SHARD, D], f32, kind="ExternalInput")
    cg_d = nc.dram_tensor("cg", [B, D], f32, kind="ExternalInput")
    f_d = nc.dram_tensor("feat", [B, D], f32, kind="ExternalInput")
    labr_d = nc.dram_tensor("labr", [1, B], f32, kind="ExternalInput")
    labc_d = nc.dram_tensor("labc", [P, NB], f32, kind="ExternalInput")
    out_shard = nc.dram_tensor("out_shard", [SHARD, D], f32, kind="ExternalOutput")
    nv_d = nc.dram_tensor("new_vals", [B, D], f32, kind="ExternalOutput")
    loss_d = nc.dram_tensor("loss", [1, 1], f32, kind="ExternalOutput")

    with ExitStack() as ctx:
        tc = ctx.enter_context(tile.TileContext(nc))
        big = ctx.enter_context(tc.tile_pool(name="big", bufs=1))
        scr = ctx.enter_context(tc.tile_pool(name="scr", bufs=4))
        st = ctx.enter_context(tc.tile_pool(name="st", bufs=1))
        pbig = ctx.enter_context(tc.tile_pool(name="pbig", bufs=4, space="PSUM"))
        psml = ctx.enter_context(tc.tile_pool(name="psml", bufs=2, space="PSUM"))

        # ---- input loads FIRST on the sync HWDGE ring (FIFO per ring), so
        # the compute chain isn't starved behind the bulk copy's packets ----
        CG = big.tile([P, NB, D], f32)      # cg     (i on partitions, d free)
        F = big.tile([P, NB, D], f32)
        labr = st.tile([1, B], f32)         # labels as f32, row layout
        labc = st.tile([P, NB], f32)        # labels as f32, (c p) layout
        nc.sync.dma_start(out=CG[:, :, :], in_=cg_d[:, :].rearrange("(c p) d -> p c d", p=P))
        nc.sync.dma_start(out=F[:, :, :], in_=f_d[:, :].rearrange("(c p) d -> p c d", p=P))
        nc.sync.dma_start(out=labr, in_=labr_d[:, :])
        nc.sync.dma_start(out=labc, in_=labc_d[:, :])

        # ---- bulk shard copy (DRAM->DRAM) behind the loads on the same
        # HWDGE ring (FIFO): loads finish first, copy then streams at the
        # ring's full rate. Splitting across rings/SWDGE was measured
        # slower (no aggregate gain + it starves the small load packets). ----
        rows = SHARD // NCHUNK
        assert rows * NCHUNK == SHARD
        for k in range(NCHUNK):
            nc.sync.dma_start(
                out=out_shard[k * rows:(k + 1) * rows, :],
                in_=shard[k * rows:(k + 1) * rows, :],
            )

        ones_p1 = nc.const_aps.tensor(1.0, (P, 1), f32)  # preamble const: dep-free
        ones_1p = st.tile([1, P], f32)
        nc.vector.memset(ones_1p, 1.0)

        # ---- cg^T on device via PE transpose (saves a 1MB HBM load) ----
        ident = st.tile([P, P], f32)
        make_identity(nc, ident)
        CT = big.tile([P, NB, D], f32)      # cg^T   (d on partitions, i free)
        CTn2 = big.tile([P, NB, D], f32)    # -2 * cg^T
        CG2 = big.tile([P, NB, D], bf16)    # bf16 cg for the E@cg matmul
        SQA = big.tile([P, NB, D], f32)     # (cg^T)^2
        CGSQ = big.tile([P, NB, D], f32)    # cg^2 (row-norm source)
        E = big.tile([P, NB, B], bf16)      # exp(-dist)*mask (symmetric)
        for c2 in range(NB):
            ptr = pbig.tile([P, B], f32, tag="mm")
            for im in range(NB):
                nc.tensor.transpose(ptr[:, im * P:(im + 1) * P],
                                    CG[:, im, c2 * P:(c2 + 1) * P], ident)
            nc.vector.tensor_copy(CT[:, c2, :], ptr)
            nc.vector.tensor_scalar(CTn2[:, c2, :], ptr, -2.0, None, op0=ALU.mult)
            nc.vector.tensor_mul(SQA[:, c2, :], CT[:, c2, :], CT[:, c2, :])
        nc.vector.tensor_copy(CG2[:, :, :], CG[:, :, :])
        nc.vector.tensor_mul(CGSQ[:, :, :], CG[:, :, :], CG[:, :, :])

        # ---- loss partials early: they only need CG/F, so DVE fills its
        # idle windows with them instead of tailing past the bulk copy ----
        ls4 = st.tile([P, NB], f32)     # per-row sum(clip((cg-f)^2))
        red2 = st.tile([P, 2], f32)
        for im in range(NB):
            df = scr.tile([P, D], f32, tag="df")
            nc.vector.tensor_sub(df, CG[:, im, :], F[:, im, :])
            sq2 = scr.tile([P, D], f32, tag="sq2")
            nc.vector.tensor_mul(sq2, df, df)
            cl = scr.tile([P, D], f32, tag="cl")
            nc.vector.tensor_scalar(cl, sq2, 1e-12, 1e12, op0=ALU.max, op1=ALU.min)
            nc.vector.tensor_reduce(ls4[:, im:im + 1], cl, AX.X, ALU.add)
        nc.vector.tensor_reduce(red2[:, 1:2], ls4, AX.X, ALU.add)
        pls = psml.tile([1, 1], f32, tag="tiny")
        nc.tensor.matmul(pls, lhsT=red2[:, 1:2], rhs=ones_p1, start=True, stop=True)
        loss_sb = st.tile([1, 1], f32)
        nc.vector.tensor_scalar(loss_sb, pls, float(SCALE / (B * D)), None, op0=ALU.mult)
        nc.gpsimd.dma_start(out=loss_d[:, :], in_=loss_sb)

        rn_i4 = st.tile([P, NB], f32)       # per-row norms, i on partitions
        for c in range(NB):
            nc.vector.tensor_reduce(rn_i4[:, c:c + 1], CGSQ[:, c, :], AX.X, ALU.add)

        # ---- rnT[1, j] = sum_d cg[j, d]^2 via ones^T @ (cg^T)^2 ----
        prnT = psml.tile([1, B], f32, tag="seq")
        for c in range(NB):
            nc.tensor.matmul(prnT, lhsT=ones_p1, rhs=SQA[:, c, :], start=(c == 0), stop=(c == NB - 1))
        rnT = st.tile([1, B], f32)
        nc.vector.tensor_copy(rnT, prnT)
        # broadcast rn_j across partitions once (K=1 matmul), keep in SBUF
        prnb = psml.tile([P, B], f32, tag="seq")
        nc.tensor.matmul(prnb, lhsT=ones_1p, rhs=rnT, start=True, stop=True)
        rnj = big.tile([P, B], f32)
        nc.vector.tensor_copy(rnj, prnb)
        # broadcast labels row across partitions (for the on-device neq mask)
        plab = psml.tile([P, B], f32, tag="seq")
        nc.tensor.matmul(plab, lhsT=ones_1p, rhs=labr, start=True, stop=True)
        labj = big.tile([P, B], f32)
        nc.vector.tensor_copy(labj, plab)

        mx4 = st.tile([P, NB], f32)     # per-row max(dist*m)  (= -min_v)
        S4 = st.tile([P, NB], f32)      # per-row sum(E)
        msr4 = st.tile([P, NB], f32)    # per-row sum(m)

        # ---- phase 1, staged across all 4 row blocks so ACT runs
        # same-function batches (2 table loads: Sqrt then Exp) ----
        pgs = []
        for im in range(NB):
            isl = slice(im * P, (im + 1) * P)
            pg = pbig.tile([P, B], f32, tag="mm")
            for dk in range(NB):
                nc.tensor.matmul(pg, lhsT=CTn2[:, dk, isl], rhs=CT[:, dk, :],
                                 start=(dk == 0), stop=(dk == NB - 1))
            pgs.append(pg)
        d2s, dists = [], []
        for im in range(NB):
            raw = scr.tile([P, B], f32, tag="raw")
            nc.vector.scalar_tensor_tensor(raw, pgs[im], rn_i4[:, im:im + 1], rnj,
                                           op0=ALU.add, op1=ALU.add)
            d2 = scr.tile([P, B], f32, tag="d2")
            nc.vector.tensor_scalar(d2, raw, 0.0, None, op0=ALU.max)
            d2s.append(d2)
        for im in range(NB):
            dist = scr.tile([P, B], f32, tag="dist")
            nc.scalar.activation(out=dist, in_=d2s[im], func=AF.Sqrt)
            dists.append(dist)
        ets = []
        for im in range(NB):
            e_t = scr.tile([P, B], f32, tag="et")
            nc.scalar.activation(out=e_t, in_=dists[im], func=AF.Exp, scale=-1.0)
            ets.append(e_t)
        for im in range(NB):
            dist = dists[im]
            le_t = scr.tile([P, B], f32, tag="le")
            nc.vector.tensor_scalar(le_t, dist, float(MARGIN), None, op0=ALU.is_le)
            m_t = scr.tile([P, B], f32, tag="mt")
            # m = (lab_j != lab_i) * (dist <= margin)
            nc.vector.scalar_tensor_tensor(m_t, labj, labc[:, im:im + 1], le_t,
                                           op0=ALU.not_equal, op1=ALU.mult)
            nc.vector.tensor_mul(E[:, im, :], ets[im], m_t)
            dm = scr.tile([P, B], f32, tag="dm")
            nc.vector.tensor_mul(dm, dist, m_t)
            nc.vector.tensor_reduce(mx4[:, im:im + 1], dm, AX.X, ALU.max)
            nc.vector.tensor_reduce(S4[:, im:im + 1], E[:, im, :], AX.X, ALU.add)
            nc.vector.tensor_reduce(msr4[:, im:im + 1], m_t, AX.X, ALU.add)

        # ---- per-row softmax factors ----
        em4 = st.tile([P, NB], f32)
        nc.scalar.activation(out=em4, in_=mx4, func=AF.Exp)
        z4 = st.tile([P, NB], f32)
        nc.vector.tensor_mul(z4, em4, S4)
        nc.vector.tensor_scalar(z4, z4, 1e-6, None, op0=ALU.add)
        rz4 = st.tile([P, NB], f32)
        nc.vector.reciprocal(rz4, z4)
        c4 = st.tile([P, NB], f32)
        nc.vector.tensor_mul(c4, em4, rz4)
        cS4 = st.tile([P, NB], f32)
        nc.vector.tensor_mul(cS4, c4, S4)

        # ---- msum gate (partition reduce via matmul) ----
        nc.vector.tensor_reduce(red2[:, 0:1], msr4, AX.X, ALU.add)
        pms = psml.tile([1, 1], f32, tag="tiny")
        nc.tensor.matmul(pms, lhsT=red2[:, 0:1], rhs=ones_p1, start=True, stop=True)
        gate = st.tile([1, 1], f32)
        nc.vector.tensor_scalar(gate, pms, 1.0, None, op0=ALU.is_ge)
        pgb = psml.tile([P, 1], f32, tag="tiny")
        nc.tensor.matmul(pgb, lhsT=ones_1p, rhs=gate, start=True, stop=True)
        gb = st.tile([P, 1], f32)
        nc.vector.tensor_copy(gb, pgb)

        # p1 = 0.5 - 0.1*g*cS ; p2 = 0.1*g*c  (per row)
        p14 = st.tile([P, NB], f32)
        nc.vector.tensor_scalar(p14, cS4, gb, -0.1, op0=ALU.mult, op1=ALU.mult)
        nc.vector.tensor_scalar(p14, p14, 0.5, None, op0=ALU.add)
        p24 = st.tile([P, NB], f32)
        nc.vector.tensor_scalar(p24, c4, gb, 0.1, op0=ALU.mult, op1=ALU.mult)

        # ---- phase 2: R = E @ cg (bf16) ; new_vals = p1*cg + 0.5*feat + p2*R ----
        for im in range(NB):
            isl = slice(im * P, (im + 1) * P)
            pr = pbig.tile([P, D], f32, tag="mm")
            for jk in range(NB):
                nc.tensor.matmul(pr, lhsT=E[:, jk, isl], rhs=CG2[:, jk, :],
                                 start=(jk == 0), stop=(jk == NB - 1))
            t1 = scr.tile([P, D], f32, tag="t1")
            nc.vector.tensor_scalar(t1, CG[:, im, :], p14[:, im:im + 1], None, op0=ALU.mult)
            t2 = scr.tile([P, D], f32, tag="t2")
            nc.vector.tensor_scalar(t2, F[:, im, :], 0.5, None, op0=ALU.mult)
            nc.vector.tensor_add(t1, t1, t2)
            t4 = scr.tile([P, D], f32, tag="t4")
            nc.vector.tensor_scalar(t4, pr, p24[:, im:im + 1], None, op0=ALU.mult)
            nc.vector.tensor_add(t1, t1, t4)
            nc.gpsimd.dma_start(out=nv_d[isl, :], in_=t1)

    nc.compile()
    return nc


def _ensure_ntff_hook():
    """Register antenv.axon_hooks (missing in this image) so
    run_bass_kernel_spmd(trace=True) can NTFF-profile via libaxon_pjrt."""
    import types

    if "antenv.axon_hooks" in sys.modules:
        return
    try:
        import antenv
        from trn_agent_boot.trn_boot import _ntff_profile_via_ctypes

        hook = _ntff_profile_via_ctypes("/opt/axon/libaxon_pjrt.so")
        mod = types.ModuleType("antenv.axon_hooks")
        mod._hook = hook
        mod.get_axon_ntff_profile_hook = lambda: mod._hook

        def _set(h):
            mod._hook = h

        mod.set_axon_ntff_profile_hook = _set
        sys.modules["antenv.axon_hooks"] = mod
        antenv.axon_hooks = mod
    except Exception:
        pass


def kernel(features, labels, centers):
    global _PROGRAM, LAST_RESULT
    from concourse.bass_utils import run_bass_kernel_spmd

    features = np.ascontiguousarray(np.asarray(features, dtype=np.float32))
    labels_np = np.asarray(labels).astype(np.int32)
    centers = np.ascontiguousarray(np.asarray(centers, dtype=np.float32))

    cg = np.ascontiguousarray(centers[labels_np])
    labr = labels_np.astype(np.float32).reshape(1, B)
    labc = np.ascontiguousarray(labels_np.astype(np.float32).reshape(NB, P).T)

    if _PROGRAM is None:
        _PROGRAM = _build_program()
    nc = _PROGRAM

    in_maps = [
        {
            "shard": centers[c * SHARD:(c + 1) * SHARD],
            "cg": cg,
            "feat": features,
            "labr": labr,
            "labc": labc,
        }
        for c in range(NCORES)
    ]
    trace = bool(int(os.environ.get("KERNEL_TRACE", "0") or "0"))
    if trace:
        _ensure_ntff_hook()
    res = run_bass_kernel_spmd(nc, in_maps, core_ids=list(range(NCORES)), trace=trace)
    LAST_RESULT = res

    out = np.empty_like(centers)
    for c in range(NCORES):
        out[c * SHARD:(c + 1) * SHARD] = res.results[c]["out_shard"]
    nv = res.results[0]["new_vals"]
    # scatter updated rows; duplicates resolve to the last batch occurrence
    rev = labels_np[::-1]
    u, idx = np.unique(rev, return_index=True)
    out[u] = nv[(B - 1) - idx]
    loss = np.float32(res.results[0]["loss"][0, 0])
    return loss, out


# revision 28
# speedup vs baseline: 1.1460x; 1.0103x over previous
# Trainium2 Bass kernel for CenterLoss (nn_CenterLoss_83090437308894).
#
# Strategy (per sharding hint): shard the 100000x512 centers table row-wise
# across 8 NeuronCores (12500 rows each). Each core bulk-copies its shard to
# the output (the memory-bound part: 25.6MB read + 25.6MB write of HBM per
# core) while redundantly computing the small (B,B) pairwise-center math:
#   dist^2 = rn_i + rn_j - 2*cg@cg^T        (PE matmuls)
#   E = exp(-dist) * mask                    (symmetric => reusable as lhsT)
#   delta2 = cS_i*cg_i - c_i*(E@cg)_i        (PE matmul + row scaling)
#   new_vals = 0.5*cg + 0.5*feat - 0.1*delta2, loss = mean(clip((cg-f)^2))
# Host does the cheap index work: gather cg = centers[labels] (1MB), the
# label inequality mask, and the final scatter of the <=512 updated rows
# (last-write-wins dedup) into the gathered output.
import os
import sys
import numpy as np

B, D, C = 512, 512, 100000
NCORES = 8
SHARD = C // NCORES          # 12500 rows per core
MARGIN = 50.0
SCALE = 1.0
NCHUNK = 50                  # bulk-copy chunks per core
P = 128
NB = B // P                  # 4 row blocks of 128

_PROGRAM = None
LAST_RESULT = None

for _p in ("/opt/trn_rl_repo", "/root/.axon_site/_ro/trn_rl_repo"):
    if os.path.isdir(_p) and _p not in sys.path:
        sys.path.append(_p)


def _build_program():
    from contextlib import ExitStack

    import concourse.bacc as bacc
    import concourse.tile as tile
    from concourse import mybir

    f32 = mybir.dt.float32
    AF = mybir.ActivationFunctionType
    ALU = mybir.AluOpType
    AX = mybir.AxisListType

    bf16 = mybir.dt.bfloat16
    from concourse.masks import make_identity

    nc = bacc.Bacc(None, target_bir_lowering=False)
    shard = nc.dram_tensor("shard", [SHARD, D], f32, kind="ExternalInput")
    cg_d = nc.dram_tensor("cg", [B, D], f32, kind="ExternalInput")
    f_d = nc.dram_tensor("feat", [B, D], f32, kind="ExternalInput")
    labr_d = nc.dram_tensor("labr", [1, B], f32, kind="ExternalInput")
    labc_d = nc.dram_tensor("labc", [P, NB], f32, kind="ExternalInput")
    out_shard = nc.dram_tensor("out_shard", [SHARD, D], f32, kind="ExternalOutput")
    nv_d = nc.dram_tensor("new_vals", [B, D], f32, kind="ExternalOutput")
    loss_d = nc.dram_tensor("loss", [1, 1], f32, kind="ExternalOutput")

    with ExitStack() as ctx:
        tc = ctx.enter_context(tile.TileContext(nc))
        big = ctx.enter_context(tc.tile_pool(name="big", bufs=1))
        scr = ctx.enter_context(tc.tile_pool(name="scr", bufs=4))
        st = ctx.enter_context(tc.tile_pool(name="st", bufs=1))
        pbig = ctx.enter_context(tc.tile_pool(name="pbig", bufs=4, space="PSUM"))
        psml = ctx.enter_context(tc.tile_pool(name="psml", bufs=2, space="PSUM"))

        # ---- input loads FIRST on the sync HWDGE ring (FIFO per ring), so
        # the compute chain isn't starved behind the bulk copy's packets ----
        CG = big.tile([P, NB, D], f32)      # cg     (i on partitions, d free)
        F = big.tile([P, NB, D], f32)
        labr = st.tile([1, B], f32)         # labels as f32, row layout
        labc = st.tile([P, NB], f32)        # labels as f32, (c p) layout
        nc.sync.dma_start(out=CG[:, :, :], in_=cg_d[:, :].rearrange("(c p) d -> p c d", p=P))
        nc.sync.dma_start(out=F[:, :, :], in_=f_d[:, :].rearrange("(c p) d -> p c d", p=P))
        nc.sync.dma_start(out=labr, in_=labr_d[:, :])
        nc.sync.dma_start(out=labc, in_=labc_d[:, :])

        # ---- bulk shard copy (DRAM->DRAM) behind the loads on the same
        # HWDGE ring (FIFO): loads finish first, copy then streams at the
        # ring's full rate. Splitting across rings/SWDGE was measured
        # slower (no aggregate gain + it starves the small load packets). ----
        rows = SHARD // NCHUNK
        assert rows * NCHUNK == SHARD
        for k in range(NCHUNK):
            nc.sync.dma_start(
                out=out_shard[k * rows:(k + 1) * rows, :],
                in_=shard[k * rows:(k + 1) * rows, :],
            )

        ones_p1 = nc.const_aps.tensor(1.0, (P, 1), f32)  # preamble const: dep-free
        ones_1p = st.tile([1, P], f32)
        nc.vector.memset(ones_1p, 1.0)

        # ---- cg^T on device via PE transpose (saves a 1MB HBM load) ----
        ident = st.tile([P, P], f32)
        make_identity(nc, ident)
        CT = big.tile([P, NB, D], f32)      # cg^T   (d on partitions, i free)
        CTn2 = big.tile([P, NB, D], f32)    # -2 * cg^T
        CG2 = big.tile([P, NB, D], bf16)    # bf16 cg for the E@cg matmul
        SQA = big.tile([P, NB, D], f32)     # (cg^T)^2
        CGSQ = big.tile([P, NB, D], f32)    # cg^2 (row-norm source)
        E = big.tile([P, NB, B], bf16)      # exp(-dist)*mask (symmetric)
        for c2 in range(NB):
            ptr = pbig.tile([P, B], f32, tag="mm")
            for im in range(NB):
                nc.tensor.transpose(ptr[:, im * P:(im + 1) * P],
                                    CG[:, im, c2 * P:(c2 + 1) * P], ident)
            nc.vector.tensor_copy(CT[:, c2, :], ptr)
            nc.vector.tensor_scalar(CTn2[:, c2, :], ptr, -2.0, None, op0=ALU.mult)
            nc.vector.tensor_mul(SQA[:, c2, :], CT[:, c2, :], CT[:, c2, :])
        nc.vector.tensor_copy(CG2[:, :, :], CG[:, :, :])
        nc.vector.tensor_mul(CGSQ[:, :, :], CG[:, :, :], CG[:, :, :])

        # ---- loss partials early: they only need CG/F, so DVE fills its
        # idle windows with them instead of tailing past the bulk copy ----
        ls4 = st.tile([P, NB], f32)     # per-row sum(clip((cg-f)^2))
        red2 = st.tile([P, 2], f32)
        for im in range(NB):
            df = scr.tile([P, D], f32, tag="df")
            nc.vector.tensor_sub(df, CG[:, im, :], F[:, im, :])
            sq2 = scr.tile([P, D], f32, tag="sq2")
            nc.vector.tensor_mul(sq2, df, df)
            cl = scr.tile([P, D], f32, tag="cl")
            nc.vector.tensor_scalar(cl, sq2, 1e-12, 1e12, op0=ALU.max, op1=ALU.min)
            nc.vector.tensor_reduce(ls4[:, im:im + 1], cl, AX.X, ALU.add)
        nc.vector.tensor_reduce(red2[:, 1:2], ls4, AX.X, ALU.add)
        pls = psml.tile([1, 1], f32, tag="tiny")
        nc.tensor.matmul(pls, lhsT=red2[:, 1:2], rhs=ones_p1, start=True, stop=True)
        loss_sb = st.tile([1, 1], f32)
        nc.vector.tensor_scalar(loss_sb, pls, float(SCALE / (B * D)), None, op0=ALU.mult)
        nc.gpsimd.dma_start(out=loss_d[:, :], in_=loss_sb)

        rn_i4 = st.tile([P, NB], f32)       # per-row norms, i on partitions
        for c in range(NB):
            nc.vector.tensor_reduce(rn_i4[:, c:c + 1], CGSQ[:, c, :], AX.X, ALU.add)

        # ---- rnT[1, j] = sum_d cg[j, d]^2 via ones^T @ (cg^T)^2 ----
        prnT = psml.tile([1, B], f32, tag="seq")
        for c in range(NB):
            nc.tensor.matmul(prnT, lhsT=ones_p1, rhs=SQA[:, c, :], start=(c == 0), stop=(c == NB - 1))
        rnT = st.tile([1, B], f32)
        nc.vector.tensor_copy(rnT, prnT)
        # broadcast rn_j across partitions once (K=1 matmul), keep in SBUF
        prnb = psml.tile([P, B], f32, tag="seq")
        nc.tensor.matmul(prnb, lhsT=ones_1p, rhs=rnT, start=True, stop=True)
        rnj = big.tile([P, B], f32)
        nc.vector.tensor_copy(rnj, prnb)
        # broadcast labels row across partitions (for the on-device neq mask)
        plab = psml.tile([P, B], f32, tag="seq")
        nc.tensor.matmul(plab, lhsT=ones_1p, rhs=labr, start=True, stop=True)
        labj = big.tile([P, B], f32)
        nc.vector.tensor_copy(labj, plab)

        mx4 = st.tile([P, NB], f32)     # per-row max(dist*m)  (= -min_v)
        S4 = st.tile([P, NB], f32)      # per-row sum(E)
        msr4 = st.tile([P, NB], f32)    # per-row sum(m)

        # ---- phase 1, staged across all 4 row blocks so ACT runs
        # same-function batches (2 table loads: Sqrt then Exp) ----
        pgs = []
        for im in range(NB):
            isl = slice(im * P, (im + 1) * P)
            pg = pbig.tile([P, B], f32, tag="mm")
            for dk in range(NB):
                nc.tensor.matmul(pg, lhsT=CTn2[:, dk, isl], rhs=CT[:, dk, :],
                                 start=(dk == 0), stop=(dk == NB - 1))
            pgs.append(pg)
        d2s, dists = [], []
        for im in range(NB):
            raw = scr.tile([P, B], f32, tag="raw")
            nc.vector.scalar_tensor_tensor(raw, pgs[im], rn_i4[:, im:im + 1], rnj,
                                           op0=ALU.add, op1=ALU.add)
            d2 = scr.tile([P, B], f32, tag="d2")
            nc.vector.tensor_scalar(d2, raw, 0.0, None, op0=ALU.max)
            d2s.append(d2)
        for im in range(NB):
            dist = scr.tile([P, B], f32, tag="dist")
            nc.scalar.activation(out=dist, in_=d2s[im], func=AF.Sqrt)
            dists.append(dist)
        ets = []
        for im in range(NB):
            e_t = scr.tile([P, B], f32, tag="et")
            nc.scalar.activation(out=e_t, in_=dists[im], func=AF.Exp, scale=-1.0)
            ets.append(e_t)
        for im in range(NB):
            dist = dists[im]
            le_t = scr.tile([P, B], f32, tag="le")
            nc.vector.tensor_scalar(le_t, dist, float(MARGIN), None, op0=ALU.is_le)
            m_t = scr.tile([P, B], f32, tag="mt")
            # m = (lab_j != lab_i) * (dist <= margin)
            nc.vector.scalar_tensor_tensor(m_t, labj, labc[:, im:im + 1], le_t,
                                           op0=ALU.not_equal, op1=ALU.mult)
            nc.vector.tensor_mul(E[:, im, :], ets[im], m_t)
            dm = scr.tile([P, B], f32, tag="dm")
            nc.vector.tensor_mul(dm, dist, m_t)
            nc.vector.tensor_reduce(mx4[:, im:im + 1], dm, AX.X, ALU.max)
            nc.vector.tensor_reduce(S4[:, im:im + 1], E[:, im, :], AX.X, ALU.add)
            nc.vector.tensor_reduce(msr4[:, im:im + 1], m_t, AX.X, ALU.add)

        # ---- per-row softmax factors ----
        em4 = st.tile([P, NB], f32)
        nc.scalar.activation(out=em4, in_=mx4, func=AF.Exp)
        z4 = st.tile([P, NB], f32)
        nc.vector.tensor_mul(z4, em4, S4)
        nc.vector.tensor_scalar(z4, z4, 1e-6, None, op0=ALU.add)
        rz4 = st.tile([P, NB], f32)
        nc.vector.reciprocal(rz4, z4)
        c4 = st.tile([P, NB], f32)
        nc.vector.tensor_mul(c4, em4, rz4)
        cS4 = st.tile([P, NB], f32)
        nc.vector.tensor_mul(cS4, c4, S4)

        # ---- msum gate (partition reduce via matmul) ----
        nc.vector.tensor_reduce(red2[:, 0:1], msr4, AX.X, ALU.add)
        pms = psml.tile([1, 1], f32, tag="tiny")
        nc.tensor.matmul(pms, lhsT=red2[:, 0:1], rhs=ones_p1, start=True, stop=True)
        gate = st.tile([1, 1], f32)
        nc.vector.tensor_scalar(gate, pms, 1.0, None, op0=ALU.is_ge)
        pgb = psml.tile([P, 1], f32, tag="tiny")
        nc.tensor.matmul(pgb, lhsT=ones_1p, rhs=gate, start=True, stop=True)
        gb = st.tile([P, 1], f32)
        nc.vector.tensor_copy(gb, pgb)

        # p1 = 0.5 - 0.1*g*cS ; p2 = 0.1*g*c  (per row)
        p14 = st.tile([P, NB], f32)
        nc.vector.tensor_scalar(p14, cS4, gb, -0.1, op0=ALU.mult, op1=ALU.mult)
        nc.vector.tensor_scalar(p14, p14, 0.5, None, op0=ALU.add)
        p24 = st.tile([P, NB], f32)
        nc.vector.tensor_scalar(p24, c4, gb, 0.1, op0=ALU.mult, op1=ALU.mult)

        # ---- phase 2: R = E @ cg (bf16) ; new_vals = p1*cg + 0.5*feat + p2*R ----
        for im in range(NB):
            isl = slice(im * P, (im + 1) * P)
            pr = pbig.tile([P, D], f32, tag="mm")
            for jk in range(NB):
                nc.tensor.matmul(pr, lhsT=E[:, jk, isl], rhs=CG2[:, jk, :],
                                 start=(jk == 0), stop=(jk == NB - 1))
            t1 = scr.tile([P, D], f32, tag="t1")
            nc.vector.tensor_scalar(t1, CG[:, im, :], p14[:, im:im + 1], None, op0=ALU.mult)
            t2 = scr.tile([P, D], f32, tag="t2")
            nc.vector.tensor_scalar(t2, F[:, im, :], 0.5, None, op0=ALU.mult)
            nc.vector.tensor_add(t1, t1, t2)
            t4 = scr.tile([P, D], f32, tag="t4")
            nc.vector.tensor_scalar(t4, pr, p24[:, im:im + 1], None, op0=ALU.mult)
            nc.vector.tensor_add(t1, t1, t4)
            nc.gpsimd.dma_start(out=nv_d[isl, :], in_=t1)

    nc.compile()
    return nc


def _ensure_ntff_hook():
    """Register antenv.axon_hooks (missing in this image) so
    run_bass_kernel_spmd(trace=True) can NTFF-profile via libaxon_pjrt."""
    import types

    if "antenv.axon_hooks" in sys.modules:
        return
    try:
        import antenv
        from trn_agent_boot.trn_boot import _ntff_profile_via_ctypes

        hook = _ntff_profile_via_ctypes("/opt/axon/libaxon_pjrt.so")
        mod = types.ModuleType("antenv.axon_hooks")
        mod._hook = hook
        mod.get_axon_ntff_profile_hook = lambda: mod._hook

        def _set(h):
            mod._hook = h

        mod.set_axon_ntff_profile_hook = _set
        sys.modules["antenv.axon_hooks"] = mod
        antenv.axon_hooks = mod
    except Exception:
        pass


def kernel(features, labels, centers):
    global _PROGRAM, LAST_RESULT
    from concourse.bass_utils import run_bass_kernel_spmd

    features = np.ascontiguousarray(np.asarray(features, dtype=np.float32))
    labels_np = np.asarray(labels).astype(np.int32)
    centers = np.ascontiguousarray(np.asarray(centers, dtype=np.float32))

    cg = np.ascontiguousarray(centers[labels_np])
    labr = labels_np.astype(np.float32).reshape(1, B)
    labc = np.ascontiguousarray(labels_np.astype(np.float32).reshape(NB, P).T)

    if _PROGRAM is None:
        _PROGRAM = _build_program()
    nc = _PROGRAM

    in_maps = [
        {
            "shard": centers[c * SHARD:(c + 1) * SHARD],
            "cg": cg,
            "feat": features,
            "labr": labr,
            "labc": labc,
        }
        for c in range(NCORES)
    ]
    trace = bool(int(os.environ.get("KERNEL_TRACE", "0") or "0"))
    if trace:
        _ensure_ntff_hook()
    res = run_bass_kernel_spmd(nc, in_maps, core_ids=list(range(NCORES)), trace=trace)
    LAST_RESULT = res

    out = np.empty_like(centers)
    for c in range(NCORES):
        out[c * SHARD:(c + 1) * SHARD] = res.results[c]["out_shard"]
    nv = res.results[0]["new_vals"]
    # scatter updated rows; duplicates resolve to the last batch occurrence
    rev = labels_np[::-1]
    u, idx = np.unique(rev, return_index=True)
    out[u] = nv[(B - 1) - idx]
    loss = np.float32(res.results[0]["loss"][0, 0])
    return loss, out
